# revision 11
# baseline (speedup 1.0000x reference)
"""Trainium2 Bass kernel for GCNN message passing.

out[b] = relu((A @ x[b]) @ W + bias),  A sparse [N, N] from 800k edges.

Sharding (8 NeuronCores): destination rows are partitioned into 200 blocks
of 250 rows each (25 blocks per core), load-balanced so every block has
nearly the same number of low-column (col < 32768) and high-column edges.
Host interleaves x into xcat[n] = x[:, n, :] (bf16, [N, 4*128]) so ONE
gather descriptor fetches a neighbor's features for all 4 batches at once.

The SWDGE descriptor-generation on the Pool engine is the bottleneck
resource (~8ns per gather index on one queue).  Two SWDGE queues are used
with alternating gathers, which pipelines descriptor generation across two
Q7 cpu pairs (~4.4ns/idx measured).

Device algorithm per core, per row-block (256-row span, 250 used):
  - two dma_gather ops (low cols on queue 0, high cols on queue 1) fetch
    msgs [128(edge), T, 512] bf16; edge slot k -> partition k%128,
    tile k//128.
  - the scaled one-hot scatter matrices S[slot, r] = (r == row[slot]) *
    val[slot] are built ON DEVICE by the (otherwise idle) Vector engine:
    S_tile = (iota == rows_scalar) * vals_scalar  via tensor_scalar,
    from compact per-slot row/val tables ([128, 25*T] f32 each).
  - PE accumulates aggT_b[c, r] += msgs[:, t, b*128:+128].T @ S_t into
    PSUM [128, 4*256] f32 (segment sum via matmul accumulation).
  - aggT -> SBUF bf16, PE applies W (outT_b = W.T @ aggT_b) into a
    second PSUM tile, ACT applies relu(.+bias), batched DMA writes
    outT [4, 128, 6400] f32.
Host scatters the per-block columns back to original row order.
"""
import sys

import numpy as np

try:  # concourse (Bass) lives in the trn repo
    import concourse  # noqa: F401
except ImportError:  # pragma: no cover
    sys.path.insert(0, "/opt/trn_rl_repo")

import ml_dtypes

B, N, E, C = 4, 50000, 800000, 128
LAST_RESULTS = None  # BassKernelResults of the most recent kernel() call
P = 128
BR = 256            # row span of a block's PSUM tile (250 rows used)
RB = 25             # row-blocks per core
NBLK = 200          # total row-blocks (8 cores x 25)
RPB = 250           # rows assigned per block (200 * 250 = 50000)
NCORES = 8
SPLIT = 32768       # low/high column split for int16 gather indices
OUT_DMA_BLKS = 4    # row-blocks per output DMA


def _balance_rows(nlo, nhi):
    """Assign each of the N rows to one of NBLK blocks (RPB rows each) so
    per-block low/high edge counts are near-uniform.

    Chunked alternating-key matching: rows sorted by total degree are
    processed in chunks of NBLK; each chunk assigns one row per block,
    pairing heavy rows with light blocks (alternating the balancing key
    between low and high counts).  Returns assignment [N] -> block id.
    """
    order = np.argsort(-(nlo + nhi), kind="stable")
    blk_lo = np.zeros(NBLK, np.int64)
    blk_hi = np.zeros(NBLK, np.int64)
    assignment = np.empty(N, np.int32)
    for i in range(RPB):
        idx = order[i * NBLK:(i + 1) * NBLK]
        if i % 2 == 0:
            rsort = idx[np.argsort(-nlo[idx], kind="stable")]
            bsort = np.argsort(blk_lo, kind="stable")
        else:
            rsort = idx[np.argsort(-nhi[idx], kind="stable")]
            bsort = np.argsort(blk_hi, kind="stable")
        assignment[rsort] = bsort
        blk_lo[bsort] += nlo[rsort]
        blk_hi[bsort] += nhi[rsort]
    return assignment, int(blk_lo.max()), int(blk_hi.max())


def _pack_idx_blocks(vals_per_block, n_slots):
    """Pack per-block int16 index vectors [nblk, n_slots] into the SWDGE
    layout: index k at [k % 16, k // 16], replicated to 128 partitions.
    Returns [128, nblk * (n_slots // 16)]."""
    nblk = vals_per_block.shape[0]
    t16 = vals_per_block.reshape(nblk, n_slots // 16, 16).transpose(0, 2, 1)
    t128 = np.tile(t16, (1, 8, 1))              # [nblk, 128, n_slots//16]
    return np.ascontiguousarray(
        t128.transpose(1, 0, 2).reshape(P, nblk * (n_slots // 16)))


def _preprocess(edge_row, edge_col, edge_vals):
    """Balanced block partition + per-core gather index tables and compact
    per-slot (row, val) tables for the on-device S build."""
    is_hi = edge_col >= SPLIT
    nlo = np.bincount(edge_row[~is_hi], minlength=N)
    nhi = np.bincount(edge_row[is_hi], minlength=N)
    assignment, max_lo, max_hi = _balance_rows(nlo, nhi)

    L = (max_lo + P - 1) // P
    H = (max_hi + P - 1) // P
    T = L + H

    # local row index within block: position in the block's row list
    perm = np.argsort(assignment, kind="stable")     # rows grouped by block
    block_rows = perm.reshape(NBLK, RPB)             # [block, local] -> row
    rloc = np.empty(N, np.int32)
    rloc[perm] = np.tile(np.arange(RPB, dtype=np.int32), NBLK)

    eblk = assignment[edge_row]                      # block of each edge
    erloc = rloc[edge_row]
    order = np.lexsort((edge_col, is_hi, eblk))
    eb, ehi, ec, er, ev = (eblk[order], is_hi[order], edge_col[order],
                           erloc[order], edge_vals[order])

    # slot within (block, group): cumcount via group-start offsets
    gkey = eb.astype(np.int64) * 2 + ehi
    starts = np.searchsorted(gkey, np.arange(NBLK * 2))
    slot = np.arange(E) - starts[gkey]
    gslot = np.where(ehi, L * P + slot, slot)        # slot within block
    tile = gslot // P
    part = gslot % P

    lowidx = np.zeros((NBLK, L * P), np.int16)
    highidx = np.zeros((NBLK, H * P), np.int16)
    lowidx[eb[~ehi], slot[~ehi]] = ec[~ehi].astype(np.int16)
    highidx[eb[ehi], slot[ehi]] = (ec[ehi] - SPLIT).astype(np.int16)

    rowsv = np.zeros((NBLK, P, T), ml_dtypes.bfloat16)
    valsv = np.zeros((NBLK, P, T), ml_dtypes.bfloat16)
    rowsv[eb, part, tile] = er.astype(ml_dtypes.bfloat16)
    valsv[eb, part, tile] = ev.astype(ml_dtypes.bfloat16)

    per_core = []
    for h in range(NCORES):
        s = slice(h * RB, (h + 1) * RB)
        per_core.append({
            "lowidx": _pack_idx_blocks(lowidx[s], L * P),
            "highidx": _pack_idx_blocks(highidx[s], H * P),
            "rowsv": np.ascontiguousarray(
                rowsv[s].transpose(1, 0, 2).reshape(P, RB * T)),
            "valsv": np.ascontiguousarray(
                valsv[s].transpose(1, 0, 2).reshape(P, RB * T)),
        })
    return per_core, block_rows, L, H


def _build_program(L, H):
    import concourse.bacc as bacc
    import concourse.tile as tile
    from concourse import mybir
    from concourse._compat import get_trn_type

    T = L + H
    BC = B * C                       # 512 feature cols in xcat
    f32 = mybir.dt.float32
    bf16 = mybir.dt.bfloat16
    i16 = mybir.dt.int16
    nc = bacc.Bacc(get_trn_type() or "TRN2", target_bir_lowering=False,
                   num_swdge_queues=2)

    x_d = nc.dram_tensor("xcat", [N, BC], bf16, kind="ExternalInput")
    lowidx_d = nc.dram_tensor("lowidx", [P, RB * 8 * L], i16,
                              kind="ExternalInput")
    highidx_d = nc.dram_tensor("highidx", [P, RB * 8 * H], i16,
                               kind="ExternalInput")
    rowsv_d = nc.dram_tensor("rowsv", [P, RB * T], bf16, kind="ExternalInput")
    valsv_d = nc.dram_tensor("valsv", [P, RB * T], bf16, kind="ExternalInput")
    iota_d = nc.dram_tensor("iota", [P, BR], bf16, kind="ExternalInput")
    wt_d = nc.dram_tensor("wt", [C, C], bf16, kind="ExternalInput")
    bias_d = nc.dram_tensor("bias", [C, 1], f32, kind="ExternalInput")
    out_d = nc.dram_tensor("outT", [B, C, RB * BR], f32,
                           kind="ExternalOutput")

    with tile.TileContext(nc) as tc:
        with (
            tc.tile_pool(name="const", bufs=1) as const_pool,
            tc.tile_pool(name="meta", bufs=1) as meta_pool,
            tc.tile_pool(name="msgs", bufs=3) as msgs_pool,
            tc.tile_pool(name="smat", bufs=2) as s_pool,
            tc.tile_pool(name="aggsb", bufs=2) as agg_pool,
            tc.tile_pool(name="ostage", bufs=2) as ostage_pool,
            tc.tile_pool(name="psum_agg", bufs=2, space="PSUM") as psA,
            tc.tile_pool(name="psum_out", bufs=2, space="PSUM") as psO,
        ):
            wt_sb = const_pool.tile([C, C], bf16)
            bias_sb = const_pool.tile([C, 1], f32)
            iota_sb = const_pool.tile([P, BR], bf16)
            nc.sync.dma_start(out=wt_sb[:], in_=wt_d[:])
            nc.sync.dma_start(out=bias_sb[:], in_=bias_d[:])
            nc.sync.dma_start(out=iota_sb[:], in_=iota_d[:])

            lowidx_sb = meta_pool.tile([P, RB * 8 * L], i16)
            highidx_sb = meta_pool.tile([P, RB * 8 * H], i16)
            rowsv_sb = meta_pool.tile([P, RB * T], bf16)
            valsv_sb = meta_pool.tile([P, RB * T], bf16)
            nc.sync.dma_start(out=lowidx_sb[:], in_=lowidx_d[:])
            nc.sync.dma_start(out=highidx_sb[:], in_=highidx_d[:])
            nc.sync.dma_start(out=rowsv_sb[:], in_=rowsv_d[:])
            nc.sync.dma_start(out=valsv_sb[:], in_=valsv_d[:])

            # split each (lo, hi) gather pair in two so the two SWDGE
            # queues carry equal descriptor-generation load per block
            La, Lb = (L + 1) // 2, L // 2
            Ha, Hb = (H + 1) // 2, H // 2
            ostage = None
            for blk in range(RB):
                msgs = msgs_pool.tile([P, T, BC], bf16)
                lo0 = blk * 8 * L
                hi0 = blk * 8 * H
                # strict q0/q1 alternation with equal per-queue tile counts
                # (q0: La+Hb, q1: Lb+Ha) so both SWDGE cpu pairs stay busy
                parts = [
                    (msgs[:, :La, :], x_d[:SPLIT, :],
                     lowidx_sb[:, lo0:lo0 + 8 * La], La, 0),
                    (msgs[:, La:L, :], x_d[:SPLIT, :],
                     lowidx_sb[:, lo0 + 8 * La:lo0 + 8 * L], Lb, 1),
                    (msgs[:, L + Ha:, :], x_d[SPLIT:, :],
                     highidx_sb[:, hi0 + 8 * Ha:hi0 + 8 * H], Hb, 0),
                    (msgs[:, L:L + Ha, :], x_d[SPLIT:, :],
                     highidx_sb[:, hi0:hi0 + 8 * Ha], Ha, 1),
                ]
                for out_ap, in_ap, idxs_ap, ntile, q in parts:
                    if ntile == 0:
                        continue
                    nc.gpsimd.dma_gather(
                        out_ap=out_ap,
                        in_ap=in_ap,
                        idxs_ap=idxs_ap,
                        num_idxs=ntile * P,
                        num_idxs_reg=ntile * P,
                        elem_size=BC,
                        single_packet=False,
                        queue_num=q,
                    )
                s_blk = s_pool.tile([P, T, BR], bf16)
                iota_brd = iota_sb[:].unsqueeze(1).broadcast_to([P, T, BR])
                rows_brd = (rowsv_sb[:, blk * T:(blk + 1) * T]
                            .unsqueeze(2).broadcast_to([P, T, BR]))
                vals_brd = (valsv_sb[:, blk * T:(blk + 1) * T]
                            .unsqueeze(2).broadcast_to([P, T, BR]))
                nc.vector.tensor_tensor(
                    out=s_blk[:], in0=iota_brd, in1=rows_brd,
                    op=mybir.AluOpType.is_equal)
                nc.vector.tensor_tensor(
                    out=s_blk[:], in0=s_blk[:], in1=vals_brd,
                    op=mybir.AluOpType.mult)
                aggT_ps = psA.tile([C, B * BR], f32)
                for bb in range(B):
                    for t in range(T):
                        nc.tensor.matmul(
                            out=aggT_ps[:, bb * BR:(bb + 1) * BR],
                            lhsT=msgs[:, t, bb * C:(bb + 1) * C],
                            rhs=s_blk[:, t, :],
                            start=(t == 0), stop=(t == T - 1),
                        )
                aggT_sb = agg_pool.tile([C, B * BR], bf16)
                nc.vector.tensor_copy(out=aggT_sb[:], in_=aggT_ps[:])
                outT_ps = psO.tile([C, B * BR], f32)
                for bb in range(B):
                    nc.tensor.matmul(
                        out=outT_ps[:, bb * BR:(bb + 1) * BR],
                        lhsT=wt_sb[:],
                        rhs=aggT_sb[:, bb * BR:(bb + 1) * BR],
                        start=True, stop=True)
                if blk % OUT_DMA_BLKS == 0:
                    ostage = ostage_pool.tile([C, B, OUT_DMA_BLKS * BR], f32)
                o_off = (blk % OUT_DMA_BLKS) * BR
                for bb in range(B):
                    nc.scalar.activation(
                        out=ostage[:, bb, o_off:o_off + BR],
                        in_=outT_ps[:, bb * BR:(bb + 1) * BR],
                        func=mybir.ActivationFunctionType.Relu,
                        bias=bias_sb[:, :1], scale=1.0,
                    )
                if blk % OUT_DMA_BLKS == OUT_DMA_BLKS - 1 or blk == RB - 1:
                    lo_blk = (blk // OUT_DMA_BLKS) * OUT_DMA_BLKS
                    width = (blk - lo_blk + 1) * BR
                    for bb in range(B):
                        nc.sync.dma_start(
                            out=out_d[bb, :, lo_blk * BR: lo_blk * BR + width],
                            in_=ostage[:, bb, :width],
                        )
    return nc


def _ensure_ntff_hook_importable():
    """bass_utils imports antenv.axon_hooks when BASS_TRACE is set; this
    image lacks that module. Provide a null hook so tracing degrades
    gracefully instead of crashing."""
    import types

    try:
        import antenv.axon_hooks  # noqa: F401
        return
    except ImportError:
        pass
    mod = types.ModuleType("antenv.axon_hooks")
    mod.get_axon_ntff_profile_hook = lambda: None
    mod.set_axon_ntff_profile_hook = lambda h: None
    sys.modules["antenv.axon_hooks"] = mod
    try:
        import antenv
        antenv.axon_hooks = mod
    except ImportError:
        pass


def kernel(x, edge_row, edge_col, edge_vals, W, b):
    _ensure_ntff_hook_importable()
    from concourse.bass_utils import run_bass_kernel_spmd

    x = np.asarray(x, np.float32)
    edge_row = np.asarray(edge_row, np.int32)
    edge_col = np.asarray(edge_col, np.int32)
    edge_vals = np.asarray(edge_vals, np.float32)
    W = np.asarray(W, np.float32)
    b = np.asarray(b, np.float32)

    per_core, block_rows, L, H = _preprocess(edge_row, edge_col, edge_vals)
    nc = _build_program(L, H)
    nc.compile()

    # xcat[n] = x[:, n, :] flattened -> [N, 4*128] bf16
    xcat = np.ascontiguousarray(
        x.transpose(1, 0, 2).reshape(N, B * C)).astype(ml_dtypes.bfloat16)
    wt = W.astype(ml_dtypes.bfloat16)
    iota = np.broadcast_to(
        np.arange(BR, dtype=np.float32)[None, :],
        (P, BR)).astype(ml_dtypes.bfloat16)
    in_maps = []
    for h in range(NCORES):
        in_maps.append({
            "xcat": xcat,
            "lowidx": per_core[h]["lowidx"],
            "highidx": per_core[h]["highidx"],
            "rowsv": per_core[h]["rowsv"],
            "valsv": per_core[h]["valsv"],
            "iota": iota,
            "wt": wt,
            "bias": np.ascontiguousarray(b[:, None]),
        })

    res = run_bass_kernel_spmd(nc, in_maps, list(range(NCORES)))
    global LAST_RESULTS
    LAST_RESULTS = res

    # columns bb*BR + i (i < RPB) of core h hold row block_rows[h*RB+bb][i]
    pos = (np.arange(RB)[:, None] * BR + np.arange(RPB)[None, :]).ravel()
    out = np.empty((B, N, C), np.float32)
    for h in range(NCORES):
        o = res.results[h]["outT"]              # [B, C, RB*BR]
        rows = block_rows[h * RB:(h + 1) * RB].ravel()
        out[:, rows, :] = o[:, :, pos].transpose(0, 2, 1)
    return out


# revision 13
# speedup vs baseline: 1.0318x; 1.0318x over previous
"""Trainium2 Bass kernel for GCNN message passing.

out[b] = relu((A @ x[b]) @ W + bias),  A sparse [N, N] from 800k edges.

Sharding (8 NeuronCores): destination rows are partitioned into 200 blocks
of 250 rows each (25 blocks per core), load-balanced so every block has
nearly the same number of low-column (col < 32768) and high-column edges.
Host interleaves x into xcat[n] = x[:, n, :] (bf16, [N, 4*128]) so ONE
gather descriptor fetches a neighbor's features for all 4 batches at once.

The SWDGE descriptor-generation on the Pool engine is the bottleneck
resource (~8ns per gather index on one queue).  Two SWDGE queues are used
with alternating gathers, which pipelines descriptor generation across two
Q7 cpu pairs (~4.4ns/idx measured).

Device algorithm per core, per row-block (256-row span, 250 used):
  - two dma_gather ops (low cols on queue 0, high cols on queue 1) fetch
    msgs [128(edge), T, 512] bf16; edge slot k -> partition k%128,
    tile k//128.
  - the scaled one-hot scatter matrices S[slot, r] = (r == row[slot]) *
    val[slot] are built ON DEVICE by the (otherwise idle) Vector engine:
    S_tile = (iota == rows_scalar) * vals_scalar  via tensor_scalar,
    from compact per-slot row/val tables ([128, 25*T] f32 each).
  - PE accumulates aggT_b[c, r] += msgs[:, t, b*128:+128].T @ S_t into
    PSUM [128, 4*256] f32 (segment sum via matmul accumulation).
  - aggT -> SBUF bf16, PE applies W (outT_b = W.T @ aggT_b) into a
    second PSUM tile, ACT applies relu(.+bias), batched DMA writes
    outT [4, 128, 6400] f32.
Host scatters the per-block columns back to original row order.
"""
import sys

import numpy as np

try:  # concourse (Bass) lives in the trn repo
    import concourse  # noqa: F401
except ImportError:  # pragma: no cover
    sys.path.insert(0, "/opt/trn_rl_repo")

import ml_dtypes

B, N, E, C = 4, 50000, 800000, 128
LAST_RESULTS = None  # BassKernelResults of the most recent kernel() call
P = 128
BR = 256            # row span of a block's PSUM tile (250 rows used)
RB = 25             # row-blocks per core
NBLK = 200          # total row-blocks (8 cores x 25)
RPB = 250           # rows assigned per block (200 * 250 = 50000)
NCORES = 8
SPLIT = 32768       # low/high column split for int16 gather indices
OUT_DMA_BLKS = 4    # row-blocks per output DMA


def _balance_rows(nlo, nhi):
    """Assign each of the N rows to one of NBLK blocks (RPB rows each) so
    per-block low/high edge counts are near-uniform.

    Chunked alternating-key matching: rows sorted by total degree are
    processed in chunks of NBLK; each chunk assigns one row per block,
    pairing heavy rows with light blocks (alternating the balancing key
    between low and high counts).  Returns assignment [N] -> block id.
    """
    order = np.argsort(-(nlo + nhi), kind="stable")
    blk_lo = np.zeros(NBLK, np.int64)
    blk_hi = np.zeros(NBLK, np.int64)
    assignment = np.empty(N, np.int32)
    for i in range(RPB):
        idx = order[i * NBLK:(i + 1) * NBLK]
        if i % 2 == 0:
            rsort = idx[np.argsort(-nlo[idx], kind="stable")]
            bsort = np.argsort(blk_lo, kind="stable")
        else:
            rsort = idx[np.argsort(-nhi[idx], kind="stable")]
            bsort = np.argsort(blk_hi, kind="stable")
        assignment[rsort] = bsort
        blk_lo[bsort] += nlo[rsort]
        blk_hi[bsort] += nhi[rsort]
    return assignment, int(blk_lo.max()), int(blk_hi.max())


def _pack_idx_blocks(vals_per_block, n_slots):
    """Pack per-block int16 index vectors [nblk, n_slots] into the SWDGE
    layout: index k at [k % 16, k // 16], replicated to 128 partitions.
    Returns [128, nblk * (n_slots // 16)]."""
    nblk = vals_per_block.shape[0]
    t16 = vals_per_block.reshape(nblk, n_slots // 16, 16).transpose(0, 2, 1)
    t128 = np.tile(t16, (1, 8, 1))              # [nblk, 128, n_slots//16]
    return np.ascontiguousarray(
        t128.transpose(1, 0, 2).reshape(P, nblk * (n_slots // 16)))


def _preprocess(edge_row, edge_col, edge_vals):
    """Balanced block partition + per-core gather index tables and compact
    per-slot (row, val) tables for the on-device S build."""
    is_hi = edge_col >= SPLIT
    nlo = np.bincount(edge_row[~is_hi], minlength=N)
    nhi = np.bincount(edge_row[is_hi], minlength=N)
    assignment, max_lo, max_hi = _balance_rows(nlo, nhi)

    L = (max_lo + P - 1) // P
    H = (max_hi + P - 1) // P
    T = L + H

    # local row index within block: position in the block's row list
    perm = np.argsort(assignment, kind="stable")     # rows grouped by block
    block_rows = perm.reshape(NBLK, RPB)             # [block, local] -> row
    rloc = np.empty(N, np.int32)
    rloc[perm] = np.tile(np.arange(RPB, dtype=np.int32), NBLK)

    eblk = assignment[edge_row]                      # block of each edge
    erloc = rloc[edge_row]
    order = np.lexsort((edge_col, is_hi, eblk))
    eb, ehi, ec, er, ev = (eblk[order], is_hi[order], edge_col[order],
                           erloc[order], edge_vals[order])

    # slot within (block, group): cumcount via group-start offsets
    gkey = eb.astype(np.int64) * 2 + ehi
    starts = np.searchsorted(gkey, np.arange(NBLK * 2))
    slot = np.arange(E) - starts[gkey]
    gslot = np.where(ehi, L * P + slot, slot)        # slot within block
    tile = gslot // P
    part = gslot % P

    lowidx = np.zeros((NBLK, L * P), np.int16)
    highidx = np.zeros((NBLK, H * P), np.int16)
    lowidx[eb[~ehi], slot[~ehi]] = ec[~ehi].astype(np.int16)
    highidx[eb[ehi], slot[ehi]] = (ec[ehi] - SPLIT).astype(np.int16)

    rowsv = np.zeros((NBLK, P, T), ml_dtypes.bfloat16)
    valsv = np.zeros((NBLK, P, T), ml_dtypes.bfloat16)
    rowsv[eb, part, tile] = er.astype(ml_dtypes.bfloat16)
    valsv[eb, part, tile] = ev.astype(ml_dtypes.bfloat16)

    per_core = []
    for h in range(NCORES):
        s = slice(h * RB, (h + 1) * RB)
        per_core.append({
            "lowidx": _pack_idx_blocks(lowidx[s], L * P),
            "highidx": _pack_idx_blocks(highidx[s], H * P),
            "rowsv": np.ascontiguousarray(
                rowsv[s].transpose(1, 0, 2).reshape(P, RB * T)),
            "valsv": np.ascontiguousarray(
                valsv[s].transpose(1, 0, 2).reshape(P, RB * T)),
        })
    return per_core, block_rows, L, H


def _build_program(L, H):
    import concourse.bacc as bacc
    import concourse.tile as tile
    from concourse import mybir
    from concourse._compat import get_trn_type

    T = L + H
    BC = B * C                       # 512 feature cols in xcat
    f32 = mybir.dt.float32
    bf16 = mybir.dt.bfloat16
    i16 = mybir.dt.int16
    nc = bacc.Bacc(get_trn_type() or "TRN2", target_bir_lowering=False,
                   num_swdge_queues=2)

    x_d = nc.dram_tensor("xcat", [N, BC], bf16, kind="ExternalInput")
    lowidx_d = nc.dram_tensor("lowidx", [P, RB * 8 * L], i16,
                              kind="ExternalInput")
    highidx_d = nc.dram_tensor("highidx", [P, RB * 8 * H], i16,
                               kind="ExternalInput")
    rowsv_d = nc.dram_tensor("rowsv", [P, RB * T], bf16, kind="ExternalInput")
    valsv_d = nc.dram_tensor("valsv", [P, RB * T], bf16, kind="ExternalInput")
    iota_d = nc.dram_tensor("iota", [P, BR], bf16, kind="ExternalInput")
    wt_d = nc.dram_tensor("wt", [C, C], bf16, kind="ExternalInput")
    bias_d = nc.dram_tensor("bias", [C, 1], f32, kind="ExternalInput")
    out_d = nc.dram_tensor("outT", [B, C, RB * BR], f32,
                           kind="ExternalOutput")

    with tile.TileContext(nc) as tc:
        with (
            tc.tile_pool(name="const", bufs=1) as const_pool,
            tc.tile_pool(name="meta", bufs=1) as meta_pool,
            tc.tile_pool(name="msgs", bufs=3) as msgs_pool,
            tc.tile_pool(name="smat", bufs=3) as s_pool,
            tc.tile_pool(name="aggsb", bufs=2) as agg_pool,
            tc.tile_pool(name="ostage", bufs=2) as ostage_pool,
            tc.tile_pool(name="psum_agg", bufs=2, space="PSUM") as psA,
            tc.tile_pool(name="psum_out", bufs=2, space="PSUM") as psO,
        ):
            wt_sb = const_pool.tile([C, C], bf16)
            bias_sb = const_pool.tile([C, 1], f32)
            iota_sb = const_pool.tile([P, BR], bf16)
            nc.sync.dma_start(out=wt_sb[:], in_=wt_d[:])
            nc.sync.dma_start(out=bias_sb[:], in_=bias_d[:])
            nc.sync.dma_start(out=iota_sb[:], in_=iota_d[:])

            lowidx_sb = meta_pool.tile([P, RB * 8 * L], i16)
            highidx_sb = meta_pool.tile([P, RB * 8 * H], i16)
            rowsv_sb = meta_pool.tile([P, RB * T], bf16)
            valsv_sb = meta_pool.tile([P, RB * T], bf16)
            nc.sync.dma_start(out=lowidx_sb[:], in_=lowidx_d[:])
            nc.sync.dma_start(out=highidx_sb[:], in_=highidx_d[:])
            nc.sync.dma_start(out=rowsv_sb[:], in_=rowsv_d[:])
            nc.sync.dma_start(out=valsv_sb[:], in_=valsv_d[:])

            # split each (lo, hi) gather pair in two so the two SWDGE
            # queues carry equal descriptor-generation load per block
            La, Lb = (L + 1) // 2, L // 2
            Ha, Hb = (H + 1) // 2, H // 2
            ostage = None
            for blk in range(RB):
                msgs = msgs_pool.tile([P, T, BC], bf16)
                lo0 = blk * 8 * L
                hi0 = blk * 8 * H
                # strict q0/q1 alternation with equal per-queue tile counts
                # (q0: La+Hb, q1: Lb+Ha) so both SWDGE cpu pairs stay busy
                parts = [
                    (msgs[:, :La, :], x_d[:SPLIT, :],
                     lowidx_sb[:, lo0:lo0 + 8 * La], La, 0),
                    (msgs[:, La:L, :], x_d[:SPLIT, :],
                     lowidx_sb[:, lo0 + 8 * La:lo0 + 8 * L], Lb, 1),
                    (msgs[:, L + Ha:, :], x_d[SPLIT:, :],
                     highidx_sb[:, hi0 + 8 * Ha:hi0 + 8 * H], Hb, 0),
                    (msgs[:, L:L + Ha, :], x_d[SPLIT:, :],
                     highidx_sb[:, hi0:hi0 + 8 * Ha], Ha, 1),
                ]
                for out_ap, in_ap, idxs_ap, ntile, q in parts:
                    if ntile == 0:
                        continue
                    nc.gpsimd.dma_gather(
                        out_ap=out_ap,
                        in_ap=in_ap,
                        idxs_ap=idxs_ap,
                        num_idxs=ntile * P,
                        num_idxs_reg=ntile * P,
                        elem_size=BC,
                        single_packet=False,
                        queue_num=q,
                    )
                s_blk = s_pool.tile([P, T, BR], bf16)
                iota_brd = iota_sb[:].unsqueeze(1).broadcast_to([P, T, BR])
                rows_brd = (rowsv_sb[:, blk * T:(blk + 1) * T]
                            .unsqueeze(2).broadcast_to([P, T, BR]))
                vals_brd = (valsv_sb[:, blk * T:(blk + 1) * T]
                            .unsqueeze(2).broadcast_to([P, T, BR]))
                nc.vector.tensor_tensor(
                    out=s_blk[:], in0=iota_brd, in1=rows_brd,
                    op=mybir.AluOpType.is_equal)
                nc.vector.tensor_tensor(
                    out=s_blk[:], in0=s_blk[:], in1=vals_brd,
                    op=mybir.AluOpType.mult)
                aggT_ps = psA.tile([C, B * BR], f32)
                for bb in range(B):
                    for t in range(T):
                        nc.tensor.matmul(
                            out=aggT_ps[:, bb * BR:(bb + 1) * BR],
                            lhsT=msgs[:, t, bb * C:(bb + 1) * C],
                            rhs=s_blk[:, t, :],
                            start=(t == 0), stop=(t == T - 1),
                        )
                aggT_sb = agg_pool.tile([C, B * BR], bf16)
                nc.scalar.activation(
                    out=aggT_sb[:], in_=aggT_ps[:],
                    func=mybir.ActivationFunctionType.Copy)
                outT_ps = psO.tile([C, B * BR], f32)
                for bb in range(B):
                    nc.tensor.matmul(
                        out=outT_ps[:, bb * BR:(bb + 1) * BR],
                        lhsT=wt_sb[:],
                        rhs=aggT_sb[:, bb * BR:(bb + 1) * BR],
                        start=True, stop=True)
                if blk % OUT_DMA_BLKS == 0:
                    ostage = ostage_pool.tile([C, B, OUT_DMA_BLKS * BR], f32)
                o_off = (blk % OUT_DMA_BLKS) * BR
                for bb in range(B):
                    nc.scalar.activation(
                        out=ostage[:, bb, o_off:o_off + BR],
                        in_=outT_ps[:, bb * BR:(bb + 1) * BR],
                        func=mybir.ActivationFunctionType.Relu,
                        bias=bias_sb[:, :1], scale=1.0,
                    )
                if blk % OUT_DMA_BLKS == OUT_DMA_BLKS - 1 or blk == RB - 1:
                    lo_blk = (blk // OUT_DMA_BLKS) * OUT_DMA_BLKS
                    width = (blk - lo_blk + 1) * BR
                    for bb in range(B):
                        nc.sync.dma_start(
                            out=out_d[bb, :, lo_blk * BR: lo_blk * BR + width],
                            in_=ostage[:, bb, :width],
                        )
    return nc


def _ensure_ntff_hook_importable():
    """bass_utils imports antenv.axon_hooks when BASS_TRACE is set; this
    image lacks that module. Provide a null hook so tracing degrades
    gracefully instead of crashing."""
    import types

    try:
        import antenv.axon_hooks  # noqa: F401
        return
    except ImportError:
        pass
    mod = types.ModuleType("antenv.axon_hooks")
    mod.get_axon_ntff_profile_hook = lambda: None
    mod.set_axon_ntff_profile_hook = lambda h: None
    sys.modules["antenv.axon_hooks"] = mod
    try:
        import antenv
        antenv.axon_hooks = mod
    except ImportError:
        pass


def kernel(x, edge_row, edge_col, edge_vals, W, b):
    _ensure_ntff_hook_importable()
    from concourse.bass_utils import run_bass_kernel_spmd

    x = np.asarray(x, np.float32)
    edge_row = np.asarray(edge_row, np.int32)
    edge_col = np.asarray(edge_col, np.int32)
    edge_vals = np.asarray(edge_vals, np.float32)
    W = np.asarray(W, np.float32)
    b = np.asarray(b, np.float32)

    per_core, block_rows, L, H = _preprocess(edge_row, edge_col, edge_vals)
    nc = _build_program(L, H)
    nc.compile()

    # xcat[n] = x[:, n, :] flattened -> [N, 4*128] bf16
    xcat = np.ascontiguousarray(
        x.transpose(1, 0, 2).reshape(N, B * C)).astype(ml_dtypes.bfloat16)
    wt = W.astype(ml_dtypes.bfloat16)
    iota = np.broadcast_to(
        np.arange(BR, dtype=np.float32)[None, :],
        (P, BR)).astype(ml_dtypes.bfloat16)
    in_maps = []
    for h in range(NCORES):
        in_maps.append({
            "xcat": xcat,
            "lowidx": per_core[h]["lowidx"],
            "highidx": per_core[h]["highidx"],
            "rowsv": per_core[h]["rowsv"],
            "valsv": per_core[h]["valsv"],
            "iota": iota,
            "wt": wt,
            "bias": np.ascontiguousarray(b[:, None]),
        })

    res = run_bass_kernel_spmd(nc, in_maps, list(range(NCORES)))
    global LAST_RESULTS
    LAST_RESULTS = res

    # columns bb*BR + i (i < RPB) of core h hold row block_rows[h*RB+bb][i]
    pos = (np.arange(RB)[:, None] * BR + np.arange(RPB)[None, :]).ravel()
    out = np.empty((B, N, C), np.float32)
    for h in range(NCORES):
        o = res.results[h]["outT"]              # [B, C, RB*BR]
        rows = block_rows[h * RB:(h + 1) * RB].ravel()
        out[:, rows, :] = o[:, :, pos].transpose(0, 2, 1)
    return out


# revision 14
# speedup vs baseline: 1.1331x; 1.0981x over previous
"""Trainium2 Bass kernel for GCNN message passing.

out[b] = relu((A @ x[b]) @ W + bias),  A sparse [N, N] from 800k edges.

Sharding (8 NeuronCores): destination rows are partitioned into 400
subblocks of 125 rows (50 per core), load-balanced so class-A subblocks
(even) and class-B subblocks (odd) carry statically known low/high-column
edge counts.  Host interleaves x into xcat[n] = x[:, n, :] (bf16,
[N, 4*128]) so ONE gather descriptor fetches a neighbor's features for
all 4 batches at once.

SWDGE descriptor generation on the Pool engine is the bottleneck resource
(~8ns/idx on one queue); gathers alternate between two SWDGE queues with
equal per-queue load, pipelining generation across two Q7 cpu pairs
(~4.4ns/idx).

Device algorithm per core, per macro-block (one A + one B subblock):
  - 4 dma_gather ops (q0: A-low + B-high, q1: B-low + A-high) fill
    msgs [128(slot), T, 512] bf16; slot k -> partition k%128, tile k//128.
  - the Vector engine builds all T scatter matrices S_t[slot, r] =
    (iota == row[slot]) * val[slot] ([128, T, 128] bf16) with two
    broadcast tensor_tensor ops from compact row/val tables.
  - PE computes agg_sub[r, (b,c)] += S_t.T @ msgs_t into PSUM
    [128, 512] f32 (S stationary, one matmul per tile, free dim 512).
  - Act copies agg to SBUF bf16; PE transposes each batch quarter
    (aggT[c, (b,r)]); Act copies back; PE applies W (W.T @ aggT);
    Act applies relu(.+bias); batched DMA writes outT [4, 128, 6250] f32.
Host scatters the per-subblock columns back to original row order.
"""
import sys

import numpy as np

try:  # concourse (Bass) lives in the trn repo
    import concourse  # noqa: F401
except ImportError:  # pragma: no cover
    sys.path.insert(0, "/opt/trn_rl_repo")

import ml_dtypes

B, N, E, C = 4, 50000, 800000, 128
LAST_RESULTS = None  # BassKernelResults of the most recent kernel() call
P = 128
SB = 128            # row span of a subblock PSUM tile (125 rows used)
RPB = 125           # rows assigned per subblock (400 * 125 = 50000)
NSUB = 400          # total subblocks (8 cores x 50)
MB = 25             # macro-blocks per core (each = A subblock + B subblock)
NCORES = 8
SPLIT = 32768       # low/high column split for int16 gather indices
OUT_DMA_BLKS = 4    # macro-blocks per output DMA


def _balance_rows(nlo, nhi, cap_lo, cap_hi):
    """Assign each of the N rows to one of NSUB subblocks (RPB rows each) so
    per-subblock low/high edge counts stay proportional to that subblock's
    class capacity (class A = even ids, class B = odd).

    Chunked alternating-key matching: rows sorted by total degree are
    processed in chunks of NSUB; each chunk assigns one row per subblock,
    pairing heavy rows with (relatively) light subblocks.
    """
    order = np.argsort(-(nlo + nhi), kind="stable")
    blk_lo = np.zeros(NSUB, np.float64)
    blk_hi = np.zeros(NSUB, np.float64)
    wlo = 1.0 / np.where(np.arange(NSUB) % 2 == 0, cap_lo[0], cap_lo[1])
    whi = 1.0 / np.where(np.arange(NSUB) % 2 == 0, cap_hi[0], cap_hi[1])
    assignment = np.empty(N, np.int32)
    for i in range(RPB):
        idx = order[i * NSUB:(i + 1) * NSUB]
        if i % 2 == 0:
            rsort = idx[np.argsort(-nlo[idx], kind="stable")]
            bsort = np.argsort(blk_lo * wlo, kind="stable")
        else:
            rsort = idx[np.argsort(-nhi[idx], kind="stable")]
            bsort = np.argsort(blk_hi * whi, kind="stable")
        assignment[rsort] = bsort
        blk_lo[bsort] += nlo[rsort]
        blk_hi[bsort] += nhi[rsort]
    lo = blk_lo.astype(np.int64)
    hi = blk_hi.astype(np.int64)
    return (assignment,
            int(lo[0::2].max()), int(lo[1::2].max()),
            int(hi[0::2].max()), int(hi[1::2].max()))


def _pack_idx_blocks(vals_per_block, n_slots):
    """Pack per-block int16 index vectors [nblk, n_slots] into the SWDGE
    layout: index k at [k % 16, k // 16], replicated to 128 partitions.
    Returns [128, nblk * (n_slots // 16)]."""
    nblk = vals_per_block.shape[0]
    t16 = vals_per_block.reshape(nblk, n_slots // 16, 16).transpose(0, 2, 1)
    t128 = np.tile(t16, (1, 8, 1))              # [nblk, 128, n_slots//16]
    return np.ascontiguousarray(
        t128.transpose(1, 0, 2).reshape(P, nblk * (n_slots // 16)))


def _preprocess(edge_row, edge_col, edge_vals):
    """Balanced subblock partition, per-core gather index tables, and
    compact per-slot (row, val) tables for the on-device S build.

    Macro-block slot order: [A-low | B-low | A-high | B-high], each group
    padded to its class tile count (LA/LB/HA/HB tiles of 128)."""
    is_hi = edge_col >= SPLIT
    nlo = np.bincount(edge_row[~is_hi], minlength=N)
    nhi = np.bincount(edge_row[is_hi], minlength=N)
    # nominal capacities steer the balancer; actual tile counts derived after
    assignment, mlo_a, mlo_b, mhi_a, mhi_b = _balance_rows(
        nlo, nhi, (11 * P, 10 * P), (6 * P, 5 * P))

    LA, LB = (mlo_a + P - 1) // P, (mlo_b + P - 1) // P
    HA, HB = (mhi_a + P - 1) // P, (mhi_b + P - 1) // P
    T = LA + LB + HA + HB
    # tile offset of each (class, is_hi) group within a macro-block
    g_off = {(0, 0): 0, (1, 0): LA, (0, 1): LA + LB, (1, 1): LA + LB + HA}
    g_ntile = {(0, 0): LA, (1, 0): LB, (0, 1): HA, (1, 1): HB}

    # local row index within subblock
    perm = np.argsort(assignment, kind="stable")
    block_rows = perm.reshape(NSUB, RPB)             # [subblock, local] -> row
    rloc = np.empty(N, np.int32)
    rloc[perm] = np.tile(np.arange(RPB, dtype=np.int32), NSUB)

    esub = assignment[edge_row]                      # subblock of each edge
    emac = esub // 2                                 # macro-block (global)
    ecls = esub % 2                                  # 0 = A, 1 = B
    erloc = rloc[edge_row]
    order = np.lexsort((edge_col, ecls, is_hi, emac))
    em, ecl, ehi, ec, er, ev = (emac[order], ecls[order], is_hi[order],
                                edge_col[order], erloc[order],
                                edge_vals[order])

    # slot within (macro, group): cumcount via group-start offsets
    gkey = (em.astype(np.int64) * 4 + ehi * 2 + ecl)
    starts = np.searchsorted(gkey, np.arange(NSUB * 2))
    slot = np.arange(E) - starts[gkey]

    goff_arr = np.empty(4, np.int64)
    gnt_arr = np.empty(4, np.int64)
    for (cls, hi), off in g_off.items():
        goff_arr[hi * 2 + cls] = off
        gnt_arr[hi * 2 + cls] = g_ntile[(cls, hi)]
    gslot = goff_arr[ehi * 2 + ecl] * P + slot       # slot within macro-block
    tile = gslot // P
    part = gslot % P

    NMAC = NSUB // 2
    lowidx = np.zeros((NMAC, (LA + LB) * P), np.int16)
    highidx = np.zeros((NMAC, (HA + HB) * P), np.int16)
    lo_m = ~ehi
    lowidx[em[lo_m], gslot[lo_m]] = ec[lo_m].astype(np.int16)
    hi_m = ehi.astype(bool)
    highidx[em[hi_m], gslot[hi_m] - (LA + LB) * P] = (
        ec[hi_m] - SPLIT).astype(np.int16)

    rowsv = np.zeros((NMAC, P, T), ml_dtypes.bfloat16)
    valsv = np.zeros((NMAC, P, T), ml_dtypes.bfloat16)
    rowsv[em, part, tile] = er.astype(ml_dtypes.bfloat16)
    valsv[em, part, tile] = ev.astype(ml_dtypes.bfloat16)

    per_core = []
    for h in range(NCORES):
        s = slice(h * MB, (h + 1) * MB)
        per_core.append({
            "lowidx": _pack_idx_blocks(lowidx[s], (LA + LB) * P),
            "highidx": _pack_idx_blocks(highidx[s], (HA + HB) * P),
            "rowsv": np.ascontiguousarray(
                rowsv[s].transpose(1, 0, 2).reshape(P, MB * T)),
            "valsv": np.ascontiguousarray(
                valsv[s].transpose(1, 0, 2).reshape(P, MB * T)),
        })
    return per_core, block_rows, (LA, LB, HA, HB)


def _build_program(LA, LB, HA, HB):
    import concourse.bacc as bacc
    import concourse.tile as tile
    from concourse import mybir
    from concourse._compat import get_trn_type

    T = LA + LB + HA + HB
    L = LA + LB
    H = HA + HB
    BC = B * C                       # 512 feature cols in xcat
    OW = OUT_DMA_BLKS * 2 * RPB      # output cols per staged DMA group
    f32 = mybir.dt.float32
    bf16 = mybir.dt.bfloat16
    i16 = mybir.dt.int16
    nc = bacc.Bacc(get_trn_type() or "TRN2", target_bir_lowering=False,
                   num_swdge_queues=2)

    x_d = nc.dram_tensor("xcat", [N, BC], bf16, kind="ExternalInput")
    lowidx_d = nc.dram_tensor("lowidx", [P, MB * 8 * L], i16,
                              kind="ExternalInput")
    highidx_d = nc.dram_tensor("highidx", [P, MB * 8 * H], i16,
                               kind="ExternalInput")
    rowsv_d = nc.dram_tensor("rowsv", [P, MB * T], bf16,
                             kind="ExternalInput")
    valsv_d = nc.dram_tensor("valsv", [P, MB * T], bf16,
                             kind="ExternalInput")
    iota_d = nc.dram_tensor("iota", [P, SB], bf16, kind="ExternalInput")
    ident_d = nc.dram_tensor("ident", [P, P], bf16, kind="ExternalInput")
    wt_d = nc.dram_tensor("wt", [C, C], bf16, kind="ExternalInput")
    bias_d = nc.dram_tensor("bias", [C, 1], f32, kind="ExternalInput")
    out_d = nc.dram_tensor("outT", [B, C, MB * 2 * RPB], f32,
                           kind="ExternalOutput")

    # tile ranges of the two subblock classes within a macro-block
    tiles_a = list(range(0, LA)) + list(range(L, L + HA))
    tiles_b = list(range(LA, L)) + list(range(L + HA, T))

    with tile.TileContext(nc) as tc:
        with (
            tc.tile_pool(name="const", bufs=1) as const_pool,
            tc.tile_pool(name="meta", bufs=1) as meta_pool,
            tc.tile_pool(name="msgs", bufs=3) as msgs_pool,
            tc.tile_pool(name="smat", bufs=3) as s_pool,
            tc.tile_pool(name="aggsb", bufs=3) as agg_pool,
            tc.tile_pool(name="aggt", bufs=3) as aggt_pool,
            tc.tile_pool(name="ostage", bufs=2) as ostage_pool,
            tc.tile_pool(name="psum_agg", bufs=4, space="PSUM") as psA,
            tc.tile_pool(name="psum_tr", bufs=2, space="PSUM") as psT,
            tc.tile_pool(name="psum_out", bufs=2, space="PSUM") as psO,
        ):
            wt_sb = const_pool.tile([C, C], bf16)
            bias_sb = const_pool.tile([C, 1], f32)
            iota_sb = const_pool.tile([P, SB], bf16)
            ident_sb = const_pool.tile([P, P], bf16)
            nc.sync.dma_start(out=wt_sb[:], in_=wt_d[:])
            nc.sync.dma_start(out=bias_sb[:], in_=bias_d[:])
            nc.sync.dma_start(out=iota_sb[:], in_=iota_d[:])
            nc.sync.dma_start(out=ident_sb[:], in_=ident_d[:])

            lowidx_sb = meta_pool.tile([P, MB * 8 * L], i16)
            highidx_sb = meta_pool.tile([P, MB * 8 * H], i16)
            rowsv_sb = meta_pool.tile([P, MB * T], bf16)
            valsv_sb = meta_pool.tile([P, MB * T], bf16)
            nc.sync.dma_start(out=lowidx_sb[:], in_=lowidx_d[:])
            nc.sync.dma_start(out=highidx_sb[:], in_=highidx_d[:])
            nc.sync.dma_start(out=rowsv_sb[:], in_=rowsv_d[:])
            nc.sync.dma_start(out=valsv_sb[:], in_=valsv_d[:])

            ostage = None
            for blk in range(MB):
                msgs = msgs_pool.tile([P, T, BC], bf16)
                lo0 = blk * 8 * L
                hi0 = blk * 8 * H
                # strict q0/q1 alternation, equal per-queue tile counts:
                # q0 = A-low + B-high, q1 = B-low + A-high
                parts = [
                    (msgs[:, :LA, :], x_d[:SPLIT, :],
                     lowidx_sb[:, lo0:lo0 + 8 * LA], LA, 0),
                    (msgs[:, LA:L, :], x_d[:SPLIT, :],
                     lowidx_sb[:, lo0 + 8 * LA:lo0 + 8 * L], LB, 1),
                    (msgs[:, L + HA:, :], x_d[SPLIT:, :],
                     highidx_sb[:, hi0 + 8 * HA:hi0 + 8 * H], HB, 0),
                    (msgs[:, L:L + HA, :], x_d[SPLIT:, :],
                     highidx_sb[:, hi0:hi0 + 8 * HA], HA, 1),
                ]
                for out_ap, in_ap, idxs_ap, ntile, q in parts:
                    nc.gpsimd.dma_gather(
                        out_ap=out_ap,
                        in_ap=in_ap,
                        idxs_ap=idxs_ap,
                        num_idxs=ntile * P,
                        num_idxs_reg=ntile * P,
                        elem_size=BC,
                        single_packet=False,
                        queue_num=q,
                    )
                s_blk = s_pool.tile([P, T, SB], bf16)
                iota_brd = iota_sb[:].unsqueeze(1).broadcast_to([P, T, SB])
                rows_brd = (rowsv_sb[:, blk * T:(blk + 1) * T]
                            .unsqueeze(2).broadcast_to([P, T, SB]))
                vals_brd = (valsv_sb[:, blk * T:(blk + 1) * T]
                            .unsqueeze(2).broadcast_to([P, T, SB]))
                nc.vector.tensor_tensor(
                    out=s_blk[:], in0=iota_brd, in1=rows_brd,
                    op=mybir.AluOpType.is_equal)
                nc.vector.tensor_tensor(
                    out=s_blk[:], in0=s_blk[:], in1=vals_brd,
                    op=mybir.AluOpType.mult)

                if blk % OUT_DMA_BLKS == 0:
                    ostage = ostage_pool.tile([C, B, OW], f32)
                for sub, tl in ((0, tiles_a), (1, tiles_b)):
                    agg_ps = psA.tile([SB, BC], f32)
                    for i, t in enumerate(tl):
                        nc.tensor.matmul(
                            out=agg_ps[:],
                            lhsT=s_blk[:, t, :],
                            rhs=msgs[:, t, :],
                            start=(i == 0), stop=(i == len(tl) - 1),
                        )
                    agg_sb = agg_pool.tile([SB, BC], bf16)
                    nc.scalar.activation(
                        out=agg_sb[:], in_=agg_ps[:],
                        func=mybir.ActivationFunctionType.Copy)
                    tr_ps = psT.tile([C, B * SB], bf16)
                    for bb in range(B):
                        nc.tensor.transpose(
                            out=tr_ps[:, bb * SB:(bb + 1) * SB],
                            in_=agg_sb[:, bb * C:(bb + 1) * C],
                            identity=ident_sb[:],
                        )
                    aggt_sb = aggt_pool.tile([C, B * SB], bf16)
                    nc.scalar.activation(
                        out=aggt_sb[:], in_=tr_ps[:],
                        func=mybir.ActivationFunctionType.Copy)
                    out_ps = psO.tile([C, B * SB], f32)
                    nc.tensor.matmul(
                        out=out_ps[:], lhsT=wt_sb[:], rhs=aggt_sb[:],
                        start=True, stop=True)
                    o_off = (blk % OUT_DMA_BLKS) * 2 * RPB + sub * RPB
                    for bb in range(B):
                        nc.scalar.activation(
                            out=ostage[:, bb, o_off:o_off + RPB],
                            in_=out_ps[:, bb * SB:bb * SB + RPB],
                            func=mybir.ActivationFunctionType.Relu,
                            bias=bias_sb[:, :1], scale=1.0,
                        )
                if blk % OUT_DMA_BLKS == OUT_DMA_BLKS - 1 or blk == MB - 1:
                    lo_blk = (blk // OUT_DMA_BLKS) * OUT_DMA_BLKS
                    width = (blk - lo_blk + 1) * 2 * RPB
                    for bb in range(B):
                        nc.sync.dma_start(
                            out=out_d[bb, :,
                                      lo_blk * 2 * RPB:
                                      lo_blk * 2 * RPB + width],
                            in_=ostage[:, bb, :width],
                        )
    return nc


def _ensure_ntff_hook_importable():
    """bass_utils imports antenv.axon_hooks when BASS_TRACE is set; this
    image lacks that module. Provide a null hook so tracing degrades
    gracefully instead of crashing."""
    import types

    try:
        import antenv.axon_hooks  # noqa: F401
        return
    except ImportError:
        pass
    mod = types.ModuleType("antenv.axon_hooks")
    mod.get_axon_ntff_profile_hook = lambda: None
    mod.set_axon_ntff_profile_hook = lambda h: None
    sys.modules["antenv.axon_hooks"] = mod
    try:
        import antenv
        antenv.axon_hooks = mod
    except ImportError:
        pass


def kernel(x, edge_row, edge_col, edge_vals, W, b):
    _ensure_ntff_hook_importable()
    from concourse.bass_utils import run_bass_kernel_spmd

    x = np.asarray(x, np.float32)
    edge_row = np.asarray(edge_row, np.int32)
    edge_col = np.asarray(edge_col, np.int32)
    edge_vals = np.asarray(edge_vals, np.float32)
    W = np.asarray(W, np.float32)
    b = np.asarray(b, np.float32)

    per_core, block_rows, (LA, LB, HA, HB) = _preprocess(
        edge_row, edge_col, edge_vals)
    nc = _build_program(LA, LB, HA, HB)
    nc.compile()

    # xcat[n] = x[:, n, :] flattened -> [N, 4*128] bf16
    xcat = np.ascontiguousarray(
        x.transpose(1, 0, 2).reshape(N, B * C)).astype(ml_dtypes.bfloat16)
    wt = W.astype(ml_dtypes.bfloat16)
    iota = np.broadcast_to(
        np.arange(SB, dtype=np.float32)[None, :],
        (P, SB)).astype(ml_dtypes.bfloat16)
    ident = np.eye(P, dtype=np.float32).astype(ml_dtypes.bfloat16)
    in_maps = []
    for h in range(NCORES):
        in_maps.append({
            "xcat": xcat,
            "lowidx": per_core[h]["lowidx"],
            "highidx": per_core[h]["highidx"],
            "rowsv": per_core[h]["rowsv"],
            "valsv": per_core[h]["valsv"],
            "iota": iota,
            "ident": ident,
            "wt": wt,
            "bias": np.ascontiguousarray(b[:, None]),
        })

    res = run_bass_kernel_spmd(nc, in_maps, list(range(NCORES)))
    global LAST_RESULTS
    LAST_RESULTS = res

    # output column sub*RPB + i of core h holds row block_rows[h*50+sub][i]
    out = np.empty((B, N, C), np.float32)
    for h in range(NCORES):
        o = res.results[h]["outT"]              # [B, C, MB*2*RPB]
        rows = block_rows[h * 50:(h + 1) * 50].ravel()
        out[:, rows, :] = o.transpose(0, 2, 1)
    return out


# revision 15
# speedup vs baseline: 1.2275x; 1.0834x over previous
"""Trainium2 Bass kernel for GCNN message passing.

out[b] = relu((A @ x[b]) @ W + bias),  A sparse [N, N] from 800k edges.

Sharding (8 NeuronCores): destination rows are partitioned into 400
subblocks of 125 rows (50 per core), load-balanced so class-A subblocks
(even) and class-B subblocks (odd) carry statically known low/high-column
edge counts.  Host interleaves x into xcat[n] = x[:, n, :] (bf16,
[N, 4*128]) so ONE gather descriptor fetches a neighbor's features for
all 4 batches at once.

SWDGE descriptor generation on the Pool engine is the bottleneck resource
(~8ns/idx on one queue); gathers alternate between two SWDGE queues with
equal per-queue load, pipelining generation across two Q7 cpu pairs
(~4.4ns/idx).

Device algorithm per core, per macro-block (one A + one B subblock):
  - 4 dma_gather ops (q0: A-low + B-high, q1: B-low + A-high) fill
    msgs [128(slot), T, 512] bf16; slot k -> partition k%128, tile k//128.
  - the Vector engine builds all T scatter matrices S_t[slot, r] =
    (iota == row[slot]) * val[slot] ([128, T, 128] bf16) with two
    broadcast tensor_tensor ops from compact row/val tables.
  - PE computes agg_sub[r, (b,c)] += S_t.T @ msgs_t into PSUM
    [128, 512] f32 (S stationary, one matmul per tile, free dim 512).
  - Act copies agg to SBUF bf16; PE transposes each batch quarter
    (aggT[c, (b,r)]); Act copies back; PE applies W (W.T @ aggT);
    Act applies relu(.+bias); batched DMA writes outT [4, 128, 6250] f32.
Host scatters the per-subblock columns back to original row order.
"""
import sys

import numpy as np

try:  # concourse (Bass) lives in the trn repo
    import concourse  # noqa: F401
except ImportError:  # pragma: no cover
    sys.path.insert(0, "/opt/trn_rl_repo")

import ml_dtypes

B, N, E, C = 4, 50000, 800000, 128
LAST_RESULTS = None  # BassKernelResults of the most recent kernel() call
P = 128
SB = 128            # row span of a subblock PSUM tile (125 rows used)
RPB = 125           # rows assigned per subblock (400 * 125 = 50000)
NSUB = 400          # total subblocks (8 cores x 50)
MB = 25             # macro-blocks per core (each = A subblock + B subblock)
NCORES = 8
SPLIT = 32768       # low/high column split for int16 gather indices
OUT_DMA_BLKS = 4    # macro-blocks per output DMA


def _balance_rows(nlo, nhi, cap_lo, cap_hi):
    """Assign each of the N rows to one of NSUB subblocks (RPB rows each) so
    per-subblock low/high edge counts stay proportional to that subblock's
    class capacity (class A = even ids, class B = odd).

    Chunked alternating-key matching: rows sorted by total degree are
    processed in chunks of NSUB; each chunk assigns one row per subblock,
    pairing heavy rows with (relatively) light subblocks.
    """
    order = np.argsort(-(nlo + nhi), kind="stable")
    blk_lo = np.zeros(NSUB, np.float64)
    blk_hi = np.zeros(NSUB, np.float64)
    wlo = 1.0 / np.where(np.arange(NSUB) % 2 == 0, cap_lo[0], cap_lo[1])
    whi = 1.0 / np.where(np.arange(NSUB) % 2 == 0, cap_hi[0], cap_hi[1])
    assignment = np.empty(N, np.int32)
    for i in range(RPB):
        idx = order[i * NSUB:(i + 1) * NSUB]
        if i % 2 == 0:
            rsort = idx[np.argsort(-nlo[idx], kind="stable")]
            bsort = np.argsort(blk_lo * wlo, kind="stable")
        else:
            rsort = idx[np.argsort(-nhi[idx], kind="stable")]
            bsort = np.argsort(blk_hi * whi, kind="stable")
        assignment[rsort] = bsort
        blk_lo[bsort] += nlo[rsort]
        blk_hi[bsort] += nhi[rsort]
    lo = blk_lo.astype(np.int64)
    hi = blk_hi.astype(np.int64)
    return (assignment,
            int(lo[0::2].max()), int(lo[1::2].max()),
            int(hi[0::2].max()), int(hi[1::2].max()))


def _pack_idx_blocks(vals_per_block, n_slots):
    """Pack per-block int16 index vectors [nblk, n_slots] into the SWDGE
    layout: index k at [k % 16, k // 16], replicated to 128 partitions.
    Returns [128, nblk * (n_slots // 16)]."""
    nblk = vals_per_block.shape[0]
    t16 = vals_per_block.reshape(nblk, n_slots // 16, 16).transpose(0, 2, 1)
    t128 = np.tile(t16, (1, 8, 1))              # [nblk, 128, n_slots//16]
    return np.ascontiguousarray(
        t128.transpose(1, 0, 2).reshape(P, nblk * (n_slots // 16)))


def _preprocess(edge_row, edge_col, edge_vals):
    """Balanced subblock partition, per-core gather index tables, and
    compact per-slot (row, val) tables for the on-device S build.

    Macro-block slot order: [A-low | B-low | A-high | B-high], each group
    padded to its class tile count (LA/LB/HA/HB tiles of 128)."""
    is_hi = edge_col >= SPLIT
    nlo = np.bincount(edge_row[~is_hi], minlength=N)
    nhi = np.bincount(edge_row[is_hi], minlength=N)
    # nominal capacities steer the balancer; actual tile counts derived after
    assignment, mlo_a, mlo_b, mhi_a, mhi_b = _balance_rows(
        nlo, nhi, (11 * P, 10 * P), (6 * P, 5 * P))

    LA, LB = (mlo_a + P - 1) // P, (mlo_b + P - 1) // P
    HA, HB = (mhi_a + P - 1) // P, (mhi_b + P - 1) // P
    T = LA + LB + HA + HB
    # tile offset of each (class, is_hi) group within a macro-block
    g_off = {(0, 0): 0, (1, 0): LA, (0, 1): LA + LB, (1, 1): LA + LB + HA}
    g_ntile = {(0, 0): LA, (1, 0): LB, (0, 1): HA, (1, 1): HB}

    # local row index within subblock
    perm = np.argsort(assignment, kind="stable")
    block_rows = perm.reshape(NSUB, RPB)             # [subblock, local] -> row
    rloc = np.empty(N, np.int32)
    rloc[perm] = np.tile(np.arange(RPB, dtype=np.int32), NSUB)

    esub = assignment[edge_row]                      # subblock of each edge
    emac = esub // 2                                 # macro-block (global)
    ecls = esub % 2                                  # 0 = A, 1 = B
    erloc = rloc[edge_row]
    order = np.lexsort((edge_col, ecls, is_hi, emac))
    em, ecl, ehi, ec, er, ev = (emac[order], ecls[order], is_hi[order],
                                edge_col[order], erloc[order],
                                edge_vals[order])

    # slot within (macro, group): cumcount via group-start offsets
    gkey = (em.astype(np.int64) * 4 + ehi * 2 + ecl)
    starts = np.searchsorted(gkey, np.arange(NSUB * 2))
    slot = np.arange(E) - starts[gkey]

    goff_arr = np.empty(4, np.int64)
    gnt_arr = np.empty(4, np.int64)
    for (cls, hi), off in g_off.items():
        goff_arr[hi * 2 + cls] = off
        gnt_arr[hi * 2 + cls] = g_ntile[(cls, hi)]
    gslot = goff_arr[ehi * 2 + ecl] * P + slot       # slot within macro-block
    tile = gslot // P
    part = gslot % P

    NMAC = NSUB // 2
    lowidx = np.zeros((NMAC, (LA + LB) * P), np.int16)
    highidx = np.zeros((NMAC, (HA + HB) * P), np.int16)
    lo_m = ~ehi
    lowidx[em[lo_m], gslot[lo_m]] = ec[lo_m].astype(np.int16)
    hi_m = ehi.astype(bool)
    highidx[em[hi_m], gslot[hi_m] - (LA + LB) * P] = (
        ec[hi_m] - SPLIT).astype(np.int16)

    rowsv = np.zeros((NMAC, P, T), ml_dtypes.bfloat16)
    valsv = np.zeros((NMAC, P, T), ml_dtypes.bfloat16)
    rowsv[em, part, tile] = er.astype(ml_dtypes.bfloat16)
    valsv[em, part, tile] = ev.astype(ml_dtypes.bfloat16)

    per_core = []
    for h in range(NCORES):
        s = slice(h * MB, (h + 1) * MB)
        per_core.append({
            "lowidx": _pack_idx_blocks(lowidx[s], (LA + LB) * P),
            "highidx": _pack_idx_blocks(highidx[s], (HA + HB) * P),
            "rowsv": np.ascontiguousarray(
                rowsv[s].transpose(1, 0, 2).reshape(P, MB * T)),
            "valsv": np.ascontiguousarray(
                valsv[s].transpose(1, 0, 2).reshape(P, MB * T)),
        })
    return per_core, block_rows, (LA, LB, HA, HB)


def _build_program(LA, LB, HA, HB):
    import concourse.bacc as bacc
    import concourse.tile as tile
    from concourse import mybir
    from concourse._compat import get_trn_type

    T = LA + LB + HA + HB
    L = LA + LB
    H = HA + HB
    BC = B * C                       # 512 feature cols in xcat
    OW = OUT_DMA_BLKS * 2 * RPB      # output cols per staged DMA group
    f32 = mybir.dt.float32
    bf16 = mybir.dt.bfloat16
    i16 = mybir.dt.int16
    nc = bacc.Bacc(get_trn_type() or "TRN2", target_bir_lowering=False,
                   num_swdge_queues=2)

    x_d = nc.dram_tensor("xcat", [N, BC], bf16, kind="ExternalInput")
    lowidx_d = nc.dram_tensor("lowidx", [P, MB * 8 * L], i16,
                              kind="ExternalInput")
    highidx_d = nc.dram_tensor("highidx", [P, MB * 8 * H], i16,
                               kind="ExternalInput")
    rowsv_d = nc.dram_tensor("rowsv", [P, MB * T], bf16,
                             kind="ExternalInput")
    valsv_d = nc.dram_tensor("valsv", [P, MB * T], bf16,
                             kind="ExternalInput")
    iota_d = nc.dram_tensor("iota", [P, SB], bf16, kind="ExternalInput")
    ident_d = nc.dram_tensor("ident", [P, P], bf16, kind="ExternalInput")
    wt_d = nc.dram_tensor("wt", [C, C], bf16, kind="ExternalInput")
    bias_d = nc.dram_tensor("bias", [C, 1], f32, kind="ExternalInput")
    out_d = nc.dram_tensor("outT", [B, C, MB * 2 * RPB], f32,
                           kind="ExternalOutput")

    # tile ranges of the two subblock classes within a macro-block
    tiles_a = list(range(0, LA)) + list(range(L, L + HA))
    tiles_b = list(range(LA, L)) + list(range(L + HA, T))

    with tile.TileContext(nc) as tc:
        with (
            tc.tile_pool(name="const", bufs=1) as const_pool,
            tc.tile_pool(name="meta", bufs=1) as meta_pool,
            tc.tile_pool(name="msgs", bufs=3) as msgs_pool,
            tc.tile_pool(name="smat", bufs=3) as s_pool,
            tc.tile_pool(name="aggsb", bufs=3) as agg_pool,
            tc.tile_pool(name="aggt", bufs=3) as aggt_pool,
            tc.tile_pool(name="ostage", bufs=2) as ostage_pool,
            tc.tile_pool(name="psum_agg", bufs=4, space="PSUM") as psA,
            tc.tile_pool(name="psum_tr", bufs=2, space="PSUM") as psT,
            tc.tile_pool(name="psum_out", bufs=2, space="PSUM") as psO,
        ):
            wt_sb = const_pool.tile([C, C], bf16)
            bias_sb = const_pool.tile([C, 1], f32)
            iota_sb = const_pool.tile([P, SB], bf16)
            ident_sb = const_pool.tile([P, P], bf16)
            nc.sync.dma_start(out=wt_sb[:], in_=wt_d[:])
            nc.sync.dma_start(out=bias_sb[:], in_=bias_d[:])
            nc.sync.dma_start(out=iota_sb[:], in_=iota_d[:])
            nc.sync.dma_start(out=ident_sb[:], in_=ident_d[:])

            lowidx_sb = meta_pool.tile([P, MB * 8 * L], i16)
            highidx_sb = meta_pool.tile([P, MB * 8 * H], i16)
            rowsv_sb = meta_pool.tile([P, MB * T], bf16)
            valsv_sb = meta_pool.tile([P, MB * T], bf16)
            nc.sync.dma_start(out=lowidx_sb[:], in_=lowidx_d[:])
            nc.sync.dma_start(out=highidx_sb[:], in_=highidx_d[:])
            nc.sync.dma_start(out=rowsv_sb[:], in_=rowsv_d[:])
            nc.sync.dma_start(out=valsv_sb[:], in_=valsv_d[:])

            ostage = None
            for blk in range(MB):
                msgs = msgs_pool.tile([P, T, BC], bf16)
                lo0 = blk * 8 * L
                hi0 = blk * 8 * H
                # one lo + one hi gather per block; queue assignment swaps
                # with block parity so each queue's serial chain carries
                # L + H tiles per block pair (balanced 2-deep pipelining)
                parts = [
                    (msgs[:, :L, :], x_d[:SPLIT, :],
                     lowidx_sb[:, lo0:lo0 + 8 * L], L, blk % 2),
                    (msgs[:, L:, :], x_d[SPLIT:, :],
                     highidx_sb[:, hi0:hi0 + 8 * H], H, 1 - blk % 2),
                ]
                for out_ap, in_ap, idxs_ap, ntile, q in parts:
                    nc.gpsimd.dma_gather(
                        out_ap=out_ap,
                        in_ap=in_ap,
                        idxs_ap=idxs_ap,
                        num_idxs=ntile * P,
                        num_idxs_reg=ntile * P,
                        elem_size=BC,
                        single_packet=False,
                        queue_num=q,
                    )
                s_blk = s_pool.tile([P, T, SB], bf16)
                iota_brd = iota_sb[:].unsqueeze(1).broadcast_to([P, T, SB])
                rows_brd = (rowsv_sb[:, blk * T:(blk + 1) * T]
                            .unsqueeze(2).broadcast_to([P, T, SB]))
                vals_brd = (valsv_sb[:, blk * T:(blk + 1) * T]
                            .unsqueeze(2).broadcast_to([P, T, SB]))
                nc.vector.tensor_tensor(
                    out=s_blk[:], in0=iota_brd, in1=rows_brd,
                    op=mybir.AluOpType.is_equal)
                nc.vector.tensor_tensor(
                    out=s_blk[:], in0=s_blk[:], in1=vals_brd,
                    op=mybir.AluOpType.mult)

                if blk % OUT_DMA_BLKS == 0:
                    ostage = ostage_pool.tile([C, B, OW], f32)
                for sub, tl in ((0, tiles_a), (1, tiles_b)):
                    agg_ps = psA.tile([SB, BC], f32)
                    for i, t in enumerate(tl):
                        nc.tensor.matmul(
                            out=agg_ps[:],
                            lhsT=s_blk[:, t, :],
                            rhs=msgs[:, t, :],
                            start=(i == 0), stop=(i == len(tl) - 1),
                        )
                    agg_sb = agg_pool.tile([SB, BC], bf16)
                    nc.scalar.activation(
                        out=agg_sb[:], in_=agg_ps[:],
                        func=mybir.ActivationFunctionType.Copy)
                    tr_ps = psT.tile([C, B * SB], bf16)
                    for bb in range(B):
                        nc.tensor.transpose(
                            out=tr_ps[:, bb * SB:(bb + 1) * SB],
                            in_=agg_sb[:, bb * C:(bb + 1) * C],
                            identity=ident_sb[:],
                        )
                    aggt_sb = aggt_pool.tile([C, B * SB], bf16)
                    nc.scalar.activation(
                        out=aggt_sb[:], in_=tr_ps[:],
                        func=mybir.ActivationFunctionType.Copy)
                    out_ps = psO.tile([C, B * SB], f32)
                    nc.tensor.matmul(
                        out=out_ps[:], lhsT=wt_sb[:], rhs=aggt_sb[:],
                        start=True, stop=True)
                    o_off = (blk % OUT_DMA_BLKS) * 2 * RPB + sub * RPB
                    for bb in range(B):
                        nc.scalar.activation(
                            out=ostage[:, bb, o_off:o_off + RPB],
                            in_=out_ps[:, bb * SB:bb * SB + RPB],
                            func=mybir.ActivationFunctionType.Relu,
                            bias=bias_sb[:, :1], scale=1.0,
                        )
                if blk % OUT_DMA_BLKS == OUT_DMA_BLKS - 1 or blk == MB - 1:
                    lo_blk = (blk // OUT_DMA_BLKS) * OUT_DMA_BLKS
                    width = (blk - lo_blk + 1) * 2 * RPB
                    for bb in range(B):
                        nc.sync.dma_start(
                            out=out_d[bb, :,
                                      lo_blk * 2 * RPB:
                                      lo_blk * 2 * RPB + width],
                            in_=ostage[:, bb, :width],
                        )
    return nc


def _ensure_ntff_hook_importable():
    """bass_utils imports antenv.axon_hooks when BASS_TRACE is set; this
    image lacks that module. Provide a null hook so tracing degrades
    gracefully instead of crashing."""
    import types

    try:
        import antenv.axon_hooks  # noqa: F401
        return
    except ImportError:
        pass
    mod = types.ModuleType("antenv.axon_hooks")
    mod.get_axon_ntff_profile_hook = lambda: None
    mod.set_axon_ntff_profile_hook = lambda h: None
    sys.modules["antenv.axon_hooks"] = mod
    try:
        import antenv
        antenv.axon_hooks = mod
    except ImportError:
        pass


def kernel(x, edge_row, edge_col, edge_vals, W, b):
    _ensure_ntff_hook_importable()
    from concourse.bass_utils import run_bass_kernel_spmd

    x = np.asarray(x, np.float32)
    edge_row = np.asarray(edge_row, np.int32)
    edge_col = np.asarray(edge_col, np.int32)
    edge_vals = np.asarray(edge_vals, np.float32)
    W = np.asarray(W, np.float32)
    b = np.asarray(b, np.float32)

    per_core, block_rows, (LA, LB, HA, HB) = _preprocess(
        edge_row, edge_col, edge_vals)
    nc = _build_program(LA, LB, HA, HB)
    nc.compile()

    # xcat[n] = x[:, n, :] flattened -> [N, 4*128] bf16
    xcat = np.ascontiguousarray(
        x.transpose(1, 0, 2).reshape(N, B * C)).astype(ml_dtypes.bfloat16)
    wt = W.astype(ml_dtypes.bfloat16)
    iota = np.broadcast_to(
        np.arange(SB, dtype=np.float32)[None, :],
        (P, SB)).astype(ml_dtypes.bfloat16)
    ident = np.eye(P, dtype=np.float32).astype(ml_dtypes.bfloat16)
    in_maps = []
    for h in range(NCORES):
        in_maps.append({
            "xcat": xcat,
            "lowidx": per_core[h]["lowidx"],
            "highidx": per_core[h]["highidx"],
            "rowsv": per_core[h]["rowsv"],
            "valsv": per_core[h]["valsv"],
            "iota": iota,
            "ident": ident,
            "wt": wt,
            "bias": np.ascontiguousarray(b[:, None]),
        })

    res = run_bass_kernel_spmd(nc, in_maps, list(range(NCORES)))
    global LAST_RESULTS
    LAST_RESULTS = res

    # output column sub*RPB + i of core h holds row block_rows[h*50+sub][i]
    out = np.empty((B, N, C), np.float32)
    for h in range(NCORES):
        o = res.results[h]["outT"]              # [B, C, MB*2*RPB]
        rows = block_rows[h * 50:(h + 1) * 50].ravel()
        out[:, rows, :] = o.transpose(0, 2, 1)
    return out


# revision 17
# speedup vs baseline: 1.3701x; 1.1161x over previous
"""Trainium2 Bass kernel for GCNN message passing.

out[b] = relu((A @ x[b]) @ W + bias),  A sparse [N, N] from 800k edges.

Sharding (8 NeuronCores): destination rows are partitioned into 400
subblocks of 125 rows (50 per core), load-balanced so class-A subblocks
(even) and class-B subblocks (odd) carry statically known low/high-column
edge counts.  Host interleaves x into xcat[n] = x[:, n, :] (bf16,
[N, 4*128]) so ONE gather descriptor fetches a neighbor's features for
all 4 batches at once.

SWDGE descriptor generation on the Pool engine is the bottleneck resource
(~8ns/idx on one queue); gathers alternate between two SWDGE queues with
equal per-queue load, pipelining generation across two Q7 cpu pairs
(~4.4ns/idx).

Device algorithm per core, per macro-block (one A + one B subblock):
  - 4 dma_gather ops (q0: A-low + B-high, q1: B-low + A-high) fill
    msgs [128(slot), T, 512] bf16; slot k -> partition k%128, tile k//128.
  - the Vector engine builds all T scatter matrices S_t[slot, r] =
    (iota == row[slot]) * val[slot] ([128, T, 128] bf16) with two
    broadcast tensor_tensor ops from compact row/val tables.
  - PE computes agg_sub[r, (b,c)] += S_t.T @ msgs_t into PSUM
    [128, 512] f32 (S stationary, one matmul per tile, free dim 512).
  - Act copies agg to SBUF bf16; PE transposes each batch quarter
    (aggT[c, (b,r)]); Act copies back; PE applies W (W.T @ aggT);
    Act applies relu(.+bias); batched DMA writes outT [4, 128, 6250] f32.
Host scatters the per-subblock columns back to original row order.
"""
import sys

import numpy as np

try:  # concourse (Bass) lives in the trn repo
    import concourse  # noqa: F401
except ImportError:  # pragma: no cover
    sys.path.insert(0, "/opt/trn_rl_repo")

import ml_dtypes

B, N, E, C = 4, 50000, 800000, 128
LAST_RESULTS = None  # BassKernelResults of the most recent kernel() call
P = 128
SB = 128            # row span of a subblock PSUM tile (125 rows used)
RPB = 125           # rows assigned per subblock (400 * 125 = 50000)
NSUB = 400          # total subblocks (8 cores x 50)
MB = 25             # macro-blocks per core (each = A subblock + B subblock)
NCORES = 8
SPLIT = 25000       # low/high column split (balances lo/hi edge counts;
                    # both halves stay addressable with int16 indices)
OUT_DMA_BLKS = 1    # macro-blocks per output DMA


def _balance_rows(nlo, nhi, cap_lo, cap_hi):
    """Assign each of the N rows to one of NSUB subblocks (RPB rows each) so
    per-subblock low/high edge counts stay proportional to that subblock's
    class capacity (class A = even ids, class B = odd).

    Chunked alternating-key matching: rows sorted by total degree are
    processed in chunks of NSUB; each chunk assigns one row per subblock,
    pairing heavy rows with (relatively) light subblocks.
    """
    order = np.argsort(-(nlo + nhi), kind="stable")
    blk_lo = np.zeros(NSUB, np.float64)
    blk_hi = np.zeros(NSUB, np.float64)
    wlo = 1.0 / np.where(np.arange(NSUB) % 2 == 0, cap_lo[0], cap_lo[1])
    whi = 1.0 / np.where(np.arange(NSUB) % 2 == 0, cap_hi[0], cap_hi[1])
    assignment = np.empty(N, np.int32)
    for i in range(RPB):
        idx = order[i * NSUB:(i + 1) * NSUB]
        if i % 2 == 0:
            rsort = idx[np.argsort(-nlo[idx], kind="stable")]
            bsort = np.argsort(blk_lo * wlo, kind="stable")
        else:
            rsort = idx[np.argsort(-nhi[idx], kind="stable")]
            bsort = np.argsort(blk_hi * whi, kind="stable")
        assignment[rsort] = bsort
        blk_lo[bsort] += nlo[rsort]
        blk_hi[bsort] += nhi[rsort]
    lo = blk_lo.astype(np.int64)
    hi = blk_hi.astype(np.int64)

    # repair pass: enforce per-class caps exactly via row swaps between the
    # most- and least-loaded bin of a class (keeps 125 rows per bin)
    rows_of = [list(np.where(assignment == b)[0]) for b in range(NSUB)]
    caps = {("lo", 0): cap_lo[0], ("lo", 1): cap_lo[1],
            ("hi", 0): cap_hi[0], ("hi", 1): cap_hi[1]}
    loads = {"lo": lo, "hi": hi}
    cnt = {"lo": nlo, "hi": nhi}
    for _ in range(2000):
        worst = None
        for (dim, cls), cap in caps.items():
            ld = loads[dim][cls::2]
            b_rel = int(np.argmax(ld))
            over = int(ld[b_rel]) - cap
            if over > 0 and (worst is None or over > worst[0]):
                worst = (over, dim, cls, cls + 2 * b_rel)
        if worst is None:
            break
        over, dim, cls, b_over = worst
        ld = loads[dim][cls::2]
        b_min = cls + 2 * int(np.argmin(ld))
        c = cnt[dim]
        r1 = max(rows_of[b_over], key=lambda r: c[r])
        r2 = min(rows_of[b_min], key=lambda r: c[r])
        if c[r1] <= c[r2]:
            break
        for d in ("lo", "hi"):
            loads[d][b_over] += cnt[d][r2] - cnt[d][r1]
            loads[d][b_min] += cnt[d][r1] - cnt[d][r2]
        rows_of[b_over].remove(r1)
        rows_of[b_min].remove(r2)
        rows_of[b_over].append(r2)
        rows_of[b_min].append(r1)
        assignment[r1] = b_min
        assignment[r2] = b_over

    return (assignment,
            int(loads["lo"][0::2].max()), int(loads["lo"][1::2].max()),
            int(loads["hi"][0::2].max()), int(loads["hi"][1::2].max()))


def _pack_idx_blocks(vals_per_block, n_slots):
    """Pack per-block int16 index vectors [nblk, n_slots] into the SWDGE
    layout: index k at [k % 16, k // 16], replicated to 128 partitions.
    Returns [128, nblk * (n_slots // 16)]."""
    nblk = vals_per_block.shape[0]
    t16 = vals_per_block.reshape(nblk, n_slots // 16, 16).transpose(0, 2, 1)
    t128 = np.tile(t16, (1, 8, 1))              # [nblk, 128, n_slots//16]
    return np.ascontiguousarray(
        t128.transpose(1, 0, 2).reshape(P, nblk * (n_slots // 16)))


def _preprocess(edge_row, edge_col, edge_vals):
    """Balanced subblock partition, per-core gather index tables, and
    compact per-slot (row, val) tables for the on-device S build.

    Macro-block slot order: [A-low | B-low | A-high | B-high], each group
    padded to its class tile count (LA/LB/HA/HB tiles of 128)."""
    is_hi = edge_col >= SPLIT
    nlo = np.bincount(edge_row[~is_hi], minlength=N)
    nhi = np.bincount(edge_row[is_hi], minlength=N)
    # nominal capacities steer the balancer; actual tile counts derived after
    assignment, mlo_a, mlo_b, mhi_a, mhi_b = _balance_rows(
        nlo, nhi, (8 * P, 8 * P), (8 * P, 8 * P))

    LA, LB = (mlo_a + P - 1) // P, (mlo_b + P - 1) // P
    HA, HB = (mhi_a + P - 1) // P, (mhi_b + P - 1) // P
    T = LA + LB + HA + HB
    # tile offset of each (class, is_hi) group within a macro-block
    g_off = {(0, 0): 0, (1, 0): LA, (0, 1): LA + LB, (1, 1): LA + LB + HA}
    g_ntile = {(0, 0): LA, (1, 0): LB, (0, 1): HA, (1, 1): HB}

    # local row index within subblock
    perm = np.argsort(assignment, kind="stable")
    block_rows = perm.reshape(NSUB, RPB)             # [subblock, local] -> row
    rloc = np.empty(N, np.int32)
    rloc[perm] = np.tile(np.arange(RPB, dtype=np.int32), NSUB)

    esub = assignment[edge_row]                      # subblock of each edge
    emac = esub // 2                                 # macro-block (global)
    ecls = esub % 2                                  # 0 = A, 1 = B
    erloc = rloc[edge_row]
    order = np.lexsort((edge_col, ecls, is_hi, emac))
    em, ecl, ehi, ec, er, ev = (emac[order], ecls[order], is_hi[order],
                                edge_col[order], erloc[order],
                                edge_vals[order])

    # slot within (macro, group): cumcount via group-start offsets
    gkey = (em.astype(np.int64) * 4 + ehi * 2 + ecl)
    starts = np.searchsorted(gkey, np.arange(NSUB * 2))
    slot = np.arange(E) - starts[gkey]

    goff_arr = np.empty(4, np.int64)
    gnt_arr = np.empty(4, np.int64)
    for (cls, hi), off in g_off.items():
        goff_arr[hi * 2 + cls] = off
        gnt_arr[hi * 2 + cls] = g_ntile[(cls, hi)]
    gslot = goff_arr[ehi * 2 + ecl] * P + slot       # slot within macro-block
    tile = gslot // P
    part = gslot % P

    NMAC = NSUB // 2
    lowidx = np.zeros((NMAC, (LA + LB) * P), np.int16)
    highidx = np.zeros((NMAC, (HA + HB) * P), np.int16)
    lo_m = ~ehi
    lowidx[em[lo_m], gslot[lo_m]] = ec[lo_m].astype(np.int16)
    hi_m = ehi.astype(bool)
    highidx[em[hi_m], gslot[hi_m] - (LA + LB) * P] = (
        ec[hi_m] - SPLIT).astype(np.int16)

    rowsv = np.zeros((NMAC, P, T), ml_dtypes.bfloat16)
    valsv = np.zeros((NMAC, P, T), ml_dtypes.bfloat16)
    rowsv[em, part, tile] = er.astype(ml_dtypes.bfloat16)
    valsv[em, part, tile] = ev.astype(ml_dtypes.bfloat16)

    per_core = []
    for h in range(NCORES):
        s = slice(h * MB, (h + 1) * MB)
        per_core.append({
            "lowidx": _pack_idx_blocks(lowidx[s], (LA + LB) * P),
            "highidx": _pack_idx_blocks(highidx[s], (HA + HB) * P),
            "rowsv": np.ascontiguousarray(
                rowsv[s].transpose(1, 0, 2).reshape(P, MB * T)),
            "valsv": np.ascontiguousarray(
                valsv[s].transpose(1, 0, 2).reshape(P, MB * T)),
        })
    return per_core, block_rows, (LA, LB, HA, HB)


def _build_program(LA, LB, HA, HB):
    import concourse.bacc as bacc
    import concourse.tile as tile
    from concourse import mybir
    from concourse._compat import get_trn_type

    T = LA + LB + HA + HB
    L = LA + LB
    H = HA + HB
    BC = B * C                       # 512 feature cols in xcat
    OW = OUT_DMA_BLKS * 2 * RPB      # output cols per staged DMA group
    f32 = mybir.dt.float32
    bf16 = mybir.dt.bfloat16
    i16 = mybir.dt.int16
    nc = bacc.Bacc(get_trn_type() or "TRN2", target_bir_lowering=False,
                   num_swdge_queues=2)

    x_d = nc.dram_tensor("xcat", [N, BC], bf16, kind="ExternalInput")
    lowidx_d = nc.dram_tensor("lowidx", [P, MB * 8 * L], i16,
                              kind="ExternalInput")
    highidx_d = nc.dram_tensor("highidx", [P, MB * 8 * H], i16,
                               kind="ExternalInput")
    rowsv_d = nc.dram_tensor("rowsv", [P, MB * T], bf16,
                             kind="ExternalInput")
    valsv_d = nc.dram_tensor("valsv", [P, MB * T], bf16,
                             kind="ExternalInput")
    iota_d = nc.dram_tensor("iota", [P, SB], bf16, kind="ExternalInput")
    ident_d = nc.dram_tensor("ident", [P, P], bf16, kind="ExternalInput")
    wt_d = nc.dram_tensor("wt", [C, C], bf16, kind="ExternalInput")
    bias_d = nc.dram_tensor("bias", [C, 1], f32, kind="ExternalInput")
    out_d = nc.dram_tensor("outT", [B, C, MB * 2 * RPB], f32,
                           kind="ExternalOutput")

    # tile ranges of the two subblock classes within a macro-block
    tiles_a = list(range(0, LA)) + list(range(L, L + HA))
    tiles_b = list(range(LA, L)) + list(range(L + HA, T))

    with tile.TileContext(nc) as tc:
        with (
            tc.tile_pool(name="const", bufs=1) as const_pool,
            tc.tile_pool(name="meta", bufs=1) as meta_pool,
            tc.tile_pool(name="msgs", bufs=4) as msgs_pool,
            tc.tile_pool(name="smat", bufs=3) as s_pool,
            tc.tile_pool(name="aggsb", bufs=3) as agg_pool,
            tc.tile_pool(name="aggt", bufs=3) as aggt_pool,
            tc.tile_pool(name="ostage", bufs=2) as ostage_pool,
            tc.tile_pool(name="psum_agg", bufs=4, space="PSUM") as psA,
            tc.tile_pool(name="psum_tr", bufs=2, space="PSUM") as psT,
            tc.tile_pool(name="psum_out", bufs=2, space="PSUM") as psO,
        ):
            wt_sb = const_pool.tile([C, C], bf16)
            bias_sb = const_pool.tile([C, 1], f32)
            iota_sb = const_pool.tile([P, SB], bf16)
            ident_sb = const_pool.tile([P, P], bf16)
            nc.sync.dma_start(out=wt_sb[:], in_=wt_d[:])
            nc.sync.dma_start(out=bias_sb[:], in_=bias_d[:])
            nc.sync.dma_start(out=iota_sb[:], in_=iota_d[:])
            nc.sync.dma_start(out=ident_sb[:], in_=ident_d[:])

            lowidx_sb = meta_pool.tile([P, MB * 8 * L], i16)
            highidx_sb = meta_pool.tile([P, MB * 8 * H], i16)
            rowsv_sb = meta_pool.tile([P, MB * T], bf16)
            valsv_sb = meta_pool.tile([P, MB * T], bf16)
            nc.sync.dma_start(out=lowidx_sb[:], in_=lowidx_d[:])
            nc.sync.dma_start(out=highidx_sb[:], in_=highidx_d[:])
            nc.sync.dma_start(out=rowsv_sb[:], in_=rowsv_d[:])
            nc.sync.dma_start(out=valsv_sb[:], in_=valsv_d[:])

            ostage = None
            for blk in range(MB):
                msgs = msgs_pool.tile([P, T, BC], bf16)
                lo0 = blk * 8 * L
                hi0 = blk * 8 * H
                # one lo + one hi gather per block; queue assignment swaps
                # with block parity so each queue's serial chain carries
                # L + H tiles per block pair (balanced 2-deep pipelining)
                parts = [
                    (msgs[:, :L, :], x_d[:SPLIT, :],
                     lowidx_sb[:, lo0:lo0 + 8 * L], L, blk % 2),
                    (msgs[:, L:, :], x_d[SPLIT:, :],
                     highidx_sb[:, hi0:hi0 + 8 * H], H, 1 - blk % 2),
                ]
                for out_ap, in_ap, idxs_ap, ntile, q in parts:
                    nc.gpsimd.dma_gather(
                        out_ap=out_ap,
                        in_ap=in_ap,
                        idxs_ap=idxs_ap,
                        num_idxs=ntile * P,
                        num_idxs_reg=ntile * P,
                        elem_size=BC,
                        single_packet=False,
                        queue_num=q,
                    )
                s_blk = s_pool.tile([P, T, SB], bf16)
                iota_brd = iota_sb[:].unsqueeze(1).broadcast_to([P, T, SB])
                rows_brd = (rowsv_sb[:, blk * T:(blk + 1) * T]
                            .unsqueeze(2).broadcast_to([P, T, SB]))
                vals_brd = (valsv_sb[:, blk * T:(blk + 1) * T]
                            .unsqueeze(2).broadcast_to([P, T, SB]))
                nc.vector.tensor_tensor(
                    out=s_blk[:], in0=iota_brd, in1=rows_brd,
                    op=mybir.AluOpType.is_equal)
                nc.vector.tensor_tensor(
                    out=s_blk[:], in0=s_blk[:], in1=vals_brd,
                    op=mybir.AluOpType.mult)

                if blk % OUT_DMA_BLKS == 0:
                    ostage = ostage_pool.tile([C, B, OW], f32)
                for sub, tl in ((0, tiles_a), (1, tiles_b)):
                    agg_ps = psA.tile([SB, BC], f32)
                    for i, t in enumerate(tl):
                        nc.tensor.matmul(
                            out=agg_ps[:],
                            lhsT=s_blk[:, t, :],
                            rhs=msgs[:, t, :],
                            start=(i == 0), stop=(i == len(tl) - 1),
                        )
                    agg_sb = agg_pool.tile([SB, BC], bf16)
                    nc.scalar.activation(
                        out=agg_sb[:], in_=agg_ps[:],
                        func=mybir.ActivationFunctionType.Copy)
                    tr_ps = psT.tile([C, B * SB], bf16)
                    for bb in range(B):
                        nc.tensor.transpose(
                            out=tr_ps[:, bb * SB:(bb + 1) * SB],
                            in_=agg_sb[:, bb * C:(bb + 1) * C],
                            identity=ident_sb[:],
                        )
                    aggt_sb = aggt_pool.tile([C, B * SB], bf16)
                    nc.scalar.activation(
                        out=aggt_sb[:], in_=tr_ps[:],
                        func=mybir.ActivationFunctionType.Copy)
                    out_ps = psO.tile([C, B * SB], f32)
                    nc.tensor.matmul(
                        out=out_ps[:], lhsT=wt_sb[:], rhs=aggt_sb[:],
                        start=True, stop=True)
                    o_off = (blk % OUT_DMA_BLKS) * 2 * RPB + sub * RPB
                    for bb in range(B):
                        nc.scalar.activation(
                            out=ostage[:, bb, o_off:o_off + RPB],
                            in_=out_ps[:, bb * SB:bb * SB + RPB],
                            func=mybir.ActivationFunctionType.Relu,
                            bias=bias_sb[:, :1], scale=1.0,
                        )
                if blk % OUT_DMA_BLKS == OUT_DMA_BLKS - 1 or blk == MB - 1:
                    lo_blk = (blk // OUT_DMA_BLKS) * OUT_DMA_BLKS
                    width = (blk - lo_blk + 1) * 2 * RPB
                    for bb in range(B):
                        nc.sync.dma_start(
                            out=out_d[bb, :,
                                      lo_blk * 2 * RPB:
                                      lo_blk * 2 * RPB + width],
                            in_=ostage[:, bb, :width],
                        )
    return nc


def _ensure_ntff_hook_importable():
    """bass_utils imports antenv.axon_hooks when BASS_TRACE is set; this
    image lacks that module. Provide a null hook so tracing degrades
    gracefully instead of crashing."""
    import types

    try:
        import antenv.axon_hooks  # noqa: F401
        return
    except ImportError:
        pass
    mod = types.ModuleType("antenv.axon_hooks")
    mod.get_axon_ntff_profile_hook = lambda: None
    mod.set_axon_ntff_profile_hook = lambda h: None
    sys.modules["antenv.axon_hooks"] = mod
    try:
        import antenv
        antenv.axon_hooks = mod
    except ImportError:
        pass


def kernel(x, edge_row, edge_col, edge_vals, W, b):
    _ensure_ntff_hook_importable()
    from concourse.bass_utils import run_bass_kernel_spmd

    x = np.asarray(x, np.float32)
    edge_row = np.asarray(edge_row, np.int32)
    edge_col = np.asarray(edge_col, np.int32)
    edge_vals = np.asarray(edge_vals, np.float32)
    W = np.asarray(W, np.float32)
    b = np.asarray(b, np.float32)

    per_core, block_rows, (LA, LB, HA, HB) = _preprocess(
        edge_row, edge_col, edge_vals)
    nc = _build_program(LA, LB, HA, HB)
    nc.compile()

    # xcat[n] = x[:, n, :] flattened -> [N, 4*128] bf16
    xcat = np.ascontiguousarray(
        x.transpose(1, 0, 2).reshape(N, B * C)).astype(ml_dtypes.bfloat16)
    wt = W.astype(ml_dtypes.bfloat16)
    iota = np.broadcast_to(
        np.arange(SB, dtype=np.float32)[None, :],
        (P, SB)).astype(ml_dtypes.bfloat16)
    ident = np.eye(P, dtype=np.float32).astype(ml_dtypes.bfloat16)
    in_maps = []
    for h in range(NCORES):
        in_maps.append({
            "xcat": xcat,
            "lowidx": per_core[h]["lowidx"],
            "highidx": per_core[h]["highidx"],
            "rowsv": per_core[h]["rowsv"],
            "valsv": per_core[h]["valsv"],
            "iota": iota,
            "ident": ident,
            "wt": wt,
            "bias": np.ascontiguousarray(b[:, None]),
        })

    res = run_bass_kernel_spmd(nc, in_maps, list(range(NCORES)))
    global LAST_RESULTS
    LAST_RESULTS = res

    # output column sub*RPB + i of core h holds row block_rows[h*50+sub][i]
    out = np.empty((B, N, C), np.float32)
    for h in range(NCORES):
        o = res.results[h]["outT"]              # [B, C, MB*2*RPB]
        rows = block_rows[h * 50:(h + 1) * 50].ravel()
        out[:, rows, :] = o.transpose(0, 2, 1)
    return out
